# revision 14
# baseline (speedup 1.0000x reference)
"""CrossAttentionPool forward on 8 TRN2 NeuronCores.

Reference computation (per batch b):
    q = lines[b] @ w_q.T ; k = videos[b] @ w_k.T
    scores = (q @ k.T) * D**-0.5, masked where video_mask==0
    out = softmax(scores, axis=-1) @ videos[b]

Strategy (data-parallel over batch, 4 batches/core):
    scores = lines @ W @ videos^T with W = (w_q.T @ w_k) * scale folded on host.
    All device tensors are fp16 (randn-scale data: fp16 keeps ~5e-4 rel err
    vs bf16's 4e-3) and the output ships fp16, halving write traffic - the
    kernel is paced by the per-core HBM share (~380 GB/s), so bytes are the
    metric that matters. Host marshalling ships lines/videos already
    transposed (feature-major), so the device runs only productive matmuls:
        u[d, 4v]    = sum_d' W[d,d'] videos[v,d']      (36 MMs, N=512)
        scores^T    = sum_d  u[d,v]  lines[l,d]        (24 MMs, N=512)
        e^T         = exp(scores^T + mask_bias[v])      (ScalarE, LUT)
        out[l,:]    = sum_v  e^T[v,l] [videos | 1 1]   (32 MMs, N=512/258)
    The two appended ones-columns give the softmax denominator in the same
    matmul; rows are scaled by its reciprocal during the PSUM->SBUF copy
    (po1 on VectorE, po2 on ScalarE - a fixed split so neither engine backs
    up the PSUM drain). The mask enters as an exp bias of -50 shipped as
    column 770 of the videos block (underflows to 0 in fp16 e^T, matching
    the reference's -1e9 masking).
    DMA discipline: only 8 distinct input DMAs (the HWDGE completion-
    semaphore pool holds 8; a 9th recycles a semaphore and its *issue*
    blocks until the earlier transfer fully lands), ordered so the
    critical u-phase operands (wl m-group 0, videos^T c-half 0) lead both
    queues. All output DMAs issue from the otherwise-idle sync engine in
    2-chunk pairs, keeping the scalar engine free for exp/scale work.
"""
import numpy as np
import concourse.bacc as bacc
import concourse.tile as tile
from concourse import mybir
from concourse.bass_utils import run_bass_kernel_spmd

N_CORES = 8
B, L, V, D = 32, 512, 128, 768
BPC = B // N_CORES          # batches per core
KC = D // 128               # 6 contraction chunks
LC = L // 128               # 4 line chunks
F32 = mybir.dt.float32
F16 = mybir.dt.float16
VW = D + 3                  # videos row | 1 1 | mask bias


def _body(tc, out_d, linesT_d, vT_d, vones_d, wl_d):
    nc = tc.nc
    from contextlib import ExitStack
    with ExitStack() as ctx:
        persist = ctx.enter_context(tc.tile_pool(name="persist", bufs=1))
        etpool = ctx.enter_context(tc.tile_pool(name="etp", bufs=3))
        outpool = ctx.enter_context(tc.tile_pool(name="osb", bufs=8))
        rpool = ctx.enter_context(tc.tile_pool(name="rp", bufs=8))

        pp_st = ctx.enter_context(tc.tile_pool(name="pp_st", bufs=2, space="PSUM"))

        # SBUF tiles (all fp16). vT: [p=d'%128, (c=d'//128, b, v)]
        vT = persist.tile([128, KC, BPC * V], F16, tag="vT")
        # wl m-major: wl_r[:, m, c, s] = WL[c*128+p, m*128+s]
        wl_r = persist.tile([128, KC, KC, 128], F16, tag="wlr")
        lT = [persist.tile([128, KC, L], F16, tag=f"lT{b}", name=f"lT{b}")
              for b in range(BPC)]
        vbr = persist.tile([128, BPC, VW], F16, tag="vbr")
        u = persist.tile([128, KC, BPC * V], F16, tag="u")

        # DMA issue order = HWDGE queue order; each queue sustains only
        # ~190 GB/s when both are active, and u's m=0 step consumes the
        # ENTIRE vT within its first 1.3 us - so the vT halves must lead
        # BOTH queues, wl m-groups alternate queues right behind (one
        # group per 1.28 us of u-pipe), then lines/vbr by need-time.
        # Sems recycle after 8 DMAs; the 9th+ gate on early transfers.
        vT_v = vT_d[:].rearrange("p (c w) -> p c w", w=BPC * V)
        wl_v = wl_d[:].rearrange("p (m c s) -> p m c s", m=KC, c=KC)
        lT_v = [linesT_d[b].rearrange("p (c w) -> p c w", w=L)
                for b in range(BPC)]
        nc.scalar.dma_start(wl_r[:, 0:1], wl_v[:, 0:1])       # s: 197 KB
        nc.sync.dma_start(vT[:, 0:3], vT_v[:, 0:3])           # y: 393 KB
        nc.scalar.dma_start(vT[:, 3:6], vT_v[:, 3:6])         # s: 393 KB
        nc.sync.dma_start(wl_r[:, 1:2], wl_v[:, 1:2])         # y: 197 KB
        nc.scalar.dma_start(wl_r[:, 2:3], wl_v[:, 2:3])       # s: 197 KB
        nc.sync.dma_start(wl_r[:, 3:4], wl_v[:, 3:4])         # y: 197 KB
        nc.scalar.dma_start(wl_r[:, 4:5], wl_v[:, 4:5])       # s: 197 KB
        nc.sync.dma_start(wl_r[:, 5:6], wl_v[:, 5:6])         # y: 197 KB
        nc.scalar.dma_start(lT[0][:], lT_v[0])                # s: 786 KB
        nc.sync.dma_start(lT[1][:], lT_v[1])                  # y: 786 KB
        nc.scalar.dma_start(vbr[:], vones_d[:].rearrange("p (b w) -> p b w",
                                                         w=VW))
        nc.sync.dma_start(lT[2][:], lT_v[2])                  # y: 786 KB
        nc.scalar.dma_start(lT[3][:], lT_v[3])                # s: 786 KB

        # ---------------- u = W @ videos^T (all 4 batches, N=512) -----------
        with tc.tile_pool(name="pp_u", bufs=2, space="PSUM") as pp_u:
            for m in range(KC):
                pu = pp_u.tile([128, BPC * V], F32)
                for c in range(KC):
                    nc.tensor.matmul(pu[:], wl_r[:, m, c], vT[:, c],
                                     start=(c == 0), stop=(c == KC - 1))
                nc.vector.tensor_copy(u[:, m], pu[:])

        # ---------------- per-batch: scores^T -> exp -> out ----------------
        with tc.tile_pool(name="pp_o1", bufs=3, space="PSUM") as pp_o1, \
             tc.tile_pool(name="pp_o2", bufs=3, space="PSUM") as pp_o2:
            for b in range(BPC):
                psT = pp_st.tile([128, 512], F32)
                for m in range(KC):
                    nc.tensor.matmul(psT[:], u[:, m, b * V:(b + 1) * V],
                                     lT[b][:, m, :],
                                     start=(m == 0), stop=(m == KC - 1))
                eT = etpool.tile([128, 512], F16)
                # one 512-col exp per batch: fewest Act-engine overheads
                nc.scalar.activation(eT[:], psT[:],
                                     mybir.ActivationFunctionType.Exp,
                                     bias=vbr[:, b, D + 2:D + 3])

                osb = None
                for i in range(LC):
                    po1 = pp_o1.tile([128, 512], F32)
                    nc.tensor.matmul(po1[:], eT[:, i * 128:(i + 1) * 128],
                                     vbr[:, b, 0:512], start=True, stop=True)
                    po2 = pp_o2.tile([128, 258], F32)
                    nc.tensor.matmul(po2[:], eT[:, i * 128:(i + 1) * 128],
                                     vbr[:, b, 512:D + 2], start=True,
                                     stop=True)
                    rec = rpool.tile([128, 1], F32)
                    nc.vector.reciprocal(rec[:], po2[:, 256:257])
                    if i % 2 == 0:
                        osb = outpool.tile([128, 2, D], F16)
                    j = i % 2
                    # PSUM->SBUF drain is the tail bottleneck; only DVE
                    # (~1.5 ns/col) and Act (~2.2 ns/col) can read PSUM,
                    # and this split balances them (~0.92 us/chunk each).
                    nc.vector.tensor_scalar_mul(osb[:, j, 0:512], po1[:],
                                                rec[:])
                    nc.scalar.mul(osb[:, j, 512:D], po2[:, 0:256], rec[:])
                    if b == BPC - 1:
                        # last batch: per-chunk DMAs so the final transfer
                        # trails the final drain by ~0.5 us, not ~1.5.
                        nc.sync.dma_start(
                            out_d[b, i * 128:(i + 1) * 128, :], osb[:, j])
                    elif j == 1:
                        dst = out_d[b, (i - 1) * 128:(i + 1) * 128, :]
                        nc.sync.dma_start(
                            dst.rearrange("(j p) d -> p j d", j=2), osb[:])


_CACHE = {}


def _build():
    if "nc" in _CACHE:
        return _CACHE["nc"]
    nc = bacc.Bacc("TRN2", target_bir_lowering=False, debug=False,
                   num_devices=N_CORES)
    linesT_d = nc.dram_tensor("linesT", [BPC, 128, KC * L], F16,
                              kind="ExternalInput").ap()
    vT_d = nc.dram_tensor("vT", [128, KC * BPC * V], F16,
                          kind="ExternalInput").ap()
    vones_d = nc.dram_tensor("vones", [128, BPC * VW], F16,
                             kind="ExternalInput").ap()
    wl_d = nc.dram_tensor("wl", [128, KC * D], F16, kind="ExternalInput").ap()
    out_d = nc.dram_tensor("out", [BPC, L, D], F16, kind="ExternalOutput").ap()
    with tile.TileContext(nc) as tc:
        _body(tc, out_d, linesT_d, vT_d, vones_d, wl_d)
    nc.compile()
    _CACHE["nc"] = nc
    return nc


def _in_maps(lines, videos, video_mask, w_q, w_k):
    w_q = np.asarray(w_q, dtype=np.float32)
    w_k = np.asarray(w_k, dtype=np.float32)
    video_mask = np.asarray(video_mask)
    scale = np.float64(D) ** -0.5
    # scores = lines @ (w_q.T @ w_k * scale) @ videos^T; device wants WL[d', d] = W[d, d']
    WL = (scale * (w_k.astype(np.float64).T @ w_q.astype(np.float64))
          ).astype(np.float32)
    mask_bias = np.where(video_mask == 0,
                         np.float32(-50.0), np.float32(0.0)).astype(np.float32)
    videos = np.asarray(videos, dtype=np.float32)
    lines = np.asarray(lines, dtype=np.float32)
    # vbr layout [v, (b, d | 1 1 | maskbias)] per core
    vones = np.concatenate(
        [videos, np.ones((B, V, 2), dtype=np.float32),
         mask_bias[:, :, None]], axis=2).astype(np.float16)
    vones = vones.reshape(N_CORES, BPC, V, VW).transpose(0, 2, 1, 3)
    vones = np.ascontiguousarray(vones.reshape(N_CORES, V, BPC * VW))
    # lT layout [b][p=d%128, (c=d//128, l)] per core
    linesT = lines.transpose(0, 2, 1).astype(np.float16)    # [B, D, L]
    linesT = linesT.reshape(B, KC, 128, L).transpose(0, 2, 1, 3)
    linesT = np.ascontiguousarray(linesT.reshape(N_CORES, BPC, 128, KC * L))
    # vT layout [p=d'%128, (c, b, v)] per core
    videosT = videos.transpose(0, 2, 1).astype(np.float16)  # [B, D, V]
    videosT = videosT.reshape(N_CORES, BPC, KC, 128, V).transpose(0, 3, 2, 1, 4)
    vT = np.ascontiguousarray(videosT.reshape(N_CORES, 128, KC * BPC * V))
    # wl layout [p, (m, c, s)] with wl[p, m, c, s] = WL[c*128+p, m*128+s]
    WLh = np.ascontiguousarray(
        WL.astype(np.float16).reshape(KC, 128, KC, 128)
        .transpose(1, 2, 0, 3).reshape(128, KC * D))
    maps = []
    for c in range(N_CORES):
        maps.append({
            "linesT": linesT[c],
            "vT": vT[c],
            "vones": vones[c],
            "wl": WLh,
        })
    return maps


def kernel(lines, videos, video_mask, w_q, w_k):
    nc = _build()
    maps = _in_maps(lines, videos, video_mask, w_q, w_k)
    res = run_bass_kernel_spmd(nc, maps, list(range(N_CORES)))
    out = np.concatenate([res.results[c]["out"] for c in range(N_CORES)], axis=0)
    return np.ascontiguousarray(out.astype(np.float32))


# revision 15
# speedup vs baseline: 1.0029x; 1.0029x over previous
"""CrossAttentionPool forward on 8 TRN2 NeuronCores.

Reference computation (per batch b):
    q = lines[b] @ w_q.T ; k = videos[b] @ w_k.T
    scores = (q @ k.T) * D**-0.5, masked where video_mask==0
    out = softmax(scores, axis=-1) @ videos[b]

Strategy (data-parallel over batch, 4 batches/core):
    scores = lines @ W @ videos^T with W = (w_q.T @ w_k) * scale folded on host.
    All device tensors are fp16 (randn-scale data: fp16 keeps ~5e-4 rel err
    vs bf16's 4e-3) and the output ships fp16, halving write traffic - the
    kernel is paced by the per-core HBM share (~380 GB/s), so bytes are the
    metric that matters. Host marshalling ships lines/videos already
    transposed (feature-major), so the device runs only productive matmuls:
        u[d, 4v]    = sum_d' W[d,d'] videos[v,d']      (36 MMs, N=512)
        scores^T    = sum_d  u[d,v]  lines[l,d]        (24 MMs, N=512)
        e^T         = exp(scores^T + mask_bias[v])      (ScalarE, LUT)
        out[l,:]    = sum_v  e^T[v,l] [videos | 1 1]   (32 MMs, N=512/258)
    The two appended ones-columns give the softmax denominator in the same
    matmul; rows are scaled by its reciprocal during the PSUM->SBUF copy
    (po1 on VectorE, po2 on ScalarE - a fixed split so neither engine backs
    up the PSUM drain). The mask enters as an exp bias of -50 shipped as
    column 770 of the videos block (underflows to 0 in fp16 e^T, matching
    the reference's -1e9 masking).
    DMA discipline: only 8 distinct input DMAs (the HWDGE completion-
    semaphore pool holds 8; a 9th recycles a semaphore and its *issue*
    blocks until the earlier transfer fully lands), ordered so the
    critical u-phase operands (wl m-group 0, videos^T c-half 0) lead both
    queues. All output DMAs issue from the otherwise-idle sync engine in
    2-chunk pairs, keeping the scalar engine free for exp/scale work.
"""
import numpy as np
import concourse.bacc as bacc
import concourse.tile as tile
from concourse import mybir
from concourse.bass_utils import run_bass_kernel_spmd

N_CORES = 8
B, L, V, D = 32, 512, 128, 768
BPC = B // N_CORES          # batches per core
KC = D // 128               # 6 contraction chunks
LC = L // 128               # 4 line chunks
F32 = mybir.dt.float32
F16 = mybir.dt.float16
VW = D + 3                  # videos row | 1 1 | mask bias


def _body(tc, out_d, linesT_d, vT_d, vones_d, wl_d):
    nc = tc.nc
    from contextlib import ExitStack
    with ExitStack() as ctx:
        persist = ctx.enter_context(tc.tile_pool(name="persist", bufs=1))
        etpool = ctx.enter_context(tc.tile_pool(name="etp", bufs=3))
        outpool = ctx.enter_context(tc.tile_pool(name="osb", bufs=8))
        rpool = ctx.enter_context(tc.tile_pool(name="rp", bufs=8))

        pp_st = ctx.enter_context(tc.tile_pool(name="pp_st", bufs=2, space="PSUM"))

        # SBUF tiles (all fp16). vT: [p=d'%128, (c=d'//128, b, v)]
        vT = persist.tile([128, KC, BPC * V], F16, tag="vT")
        # wl m-major: wl_r[:, m, c, s] = WL[c*128+p, m*128+s]
        wl_r = persist.tile([128, KC, KC, 128], F16, tag="wlr")
        lT = [persist.tile([128, KC, L], F16, tag=f"lT{b}", name=f"lT{b}")
              for b in range(BPC)]
        vbr = persist.tile([128, BPC, VW], F16, tag="vbr")
        u = persist.tile([128, KC, BPC * V], F16, tag="u")

        # DMA issue order = HWDGE queue order; each queue sustains only
        # ~190 GB/s when both are active, and u's m=0 step consumes the
        # ENTIRE vT within its first 1.3 us - so the vT halves must lead
        # BOTH queues, wl m-groups alternate queues right behind (one
        # group per 1.28 us of u-pipe), then lines/vbr by need-time.
        # Sems recycle after 8 DMAs; the 9th+ gate on early transfers.
        vT_v = vT_d[:].rearrange("p (c w) -> p c w", w=BPC * V)
        wl_v = wl_d[:].rearrange("p (m c s) -> p m c s", m=KC, c=KC)
        lT_v = [linesT_d[b].rearrange("p (c w) -> p c w", w=L)
                for b in range(BPC)]
        nc.scalar.dma_start(wl_r[:, 0:1], wl_v[:, 0:1])       # s: 197 KB
        nc.sync.dma_start(vT[:, 0:3], vT_v[:, 0:3])           # y: 393 KB
        nc.scalar.dma_start(vT[:, 3:6], vT_v[:, 3:6])         # s: 393 KB
        nc.sync.dma_start(wl_r[:, 1:2], wl_v[:, 1:2])         # y: 197 KB
        nc.scalar.dma_start(wl_r[:, 2:3], wl_v[:, 2:3])       # s: 197 KB
        nc.sync.dma_start(wl_r[:, 3:4], wl_v[:, 3:4])         # y: 197 KB
        nc.scalar.dma_start(wl_r[:, 4:5], wl_v[:, 4:5])       # s: 197 KB
        nc.sync.dma_start(wl_r[:, 5:6], wl_v[:, 5:6])         # y: 197 KB
        nc.scalar.dma_start(lT[0][:], lT_v[0])                # s: 786 KB
        nc.sync.dma_start(lT[1][:], lT_v[1])                  # y: 786 KB
        nc.scalar.dma_start(vbr[:], vones_d[:].rearrange("p (b w) -> p b w",
                                                         w=VW))
        nc.sync.dma_start(lT[2][:], lT_v[2])                  # y: 786 KB
        nc.scalar.dma_start(lT[3][:], lT_v[3])                # s: 786 KB

        # ---------------- u = W @ videos^T (all 4 batches, N=512) -----------
        with tc.tile_pool(name="pp_u", bufs=2, space="PSUM") as pp_u:
            for m in range(KC):
                pu = pp_u.tile([128, BPC * V], F32)
                for c in range(KC):
                    nc.tensor.matmul(pu[:], wl_r[:, m, c], vT[:, c],
                                     start=(c == 0), stop=(c == KC - 1))
                nc.vector.tensor_copy(u[:, m], pu[:])

        # ---------------- per-batch: scores^T -> exp -> out ----------------
        with tc.tile_pool(name="pp_o1", bufs=3, space="PSUM") as pp_o1, \
             tc.tile_pool(name="pp_o2", bufs=3, space="PSUM") as pp_o2:
            for b in range(BPC):
                psT = pp_st.tile([128, 512], F32)
                for m in range(KC):
                    nc.tensor.matmul(psT[:], u[:, m, b * V:(b + 1) * V],
                                     lT[b][:, m, :],
                                     start=(m == 0), stop=(m == KC - 1))
                eT = etpool.tile([128, 512], F16)
                # exp in 256-col slices: fewer Act-engine instruction
                # overheads; the first out-MMs start after 1/2
                for i in range(2):
                    nc.scalar.activation(eT[:, i * 256:(i + 1) * 256],
                                         psT[:, i * 256:(i + 1) * 256],
                                         mybir.ActivationFunctionType.Exp,
                                         bias=vbr[:, b, D + 2:D + 3])

                osb = None
                for i in range(LC):
                    po1 = pp_o1.tile([128, 512], F32)
                    nc.tensor.matmul(po1[:], eT[:, i * 128:(i + 1) * 128],
                                     vbr[:, b, 0:512], start=True, stop=True)
                    po2 = pp_o2.tile([128, 258], F32)
                    nc.tensor.matmul(po2[:], eT[:, i * 128:(i + 1) * 128],
                                     vbr[:, b, 512:D + 2], start=True,
                                     stop=True)
                    rec = rpool.tile([128, 1], F32)
                    nc.vector.reciprocal(rec[:], po2[:, 256:257])
                    if i % 2 == 0:
                        osb = outpool.tile([128, 2, D], F16)
                    j = i % 2
                    # PSUM->SBUF drain is the tail bottleneck; only DVE
                    # (~1.5 ns/col) and Act (~2.2 ns/col) can read PSUM,
                    # and this split balances them (~0.92 us/chunk each).
                    nc.vector.tensor_scalar_mul(osb[:, j, 0:512], po1[:],
                                                rec[:])
                    nc.scalar.mul(osb[:, j, 512:D], po2[:, 0:256], rec[:])
                    if b == BPC - 1:
                        # last batch: per-chunk DMAs so the final transfer
                        # trails the final drain by ~0.5 us, not ~1.5.
                        nc.sync.dma_start(
                            out_d[b, i * 128:(i + 1) * 128, :], osb[:, j])
                    elif j == 1:
                        dst = out_d[b, (i - 1) * 128:(i + 1) * 128, :]
                        nc.sync.dma_start(
                            dst.rearrange("(j p) d -> p j d", j=2), osb[:])


_CACHE = {}


def _build():
    if "nc" in _CACHE:
        return _CACHE["nc"]
    nc = bacc.Bacc("TRN2", target_bir_lowering=False, debug=False,
                   num_devices=N_CORES)
    linesT_d = nc.dram_tensor("linesT", [BPC, 128, KC * L], F16,
                              kind="ExternalInput").ap()
    vT_d = nc.dram_tensor("vT", [128, KC * BPC * V], F16,
                          kind="ExternalInput").ap()
    vones_d = nc.dram_tensor("vones", [128, BPC * VW], F16,
                             kind="ExternalInput").ap()
    wl_d = nc.dram_tensor("wl", [128, KC * D], F16, kind="ExternalInput").ap()
    out_d = nc.dram_tensor("out", [BPC, L, D], F16, kind="ExternalOutput").ap()
    with tile.TileContext(nc) as tc:
        _body(tc, out_d, linesT_d, vT_d, vones_d, wl_d)
    nc.compile()
    _CACHE["nc"] = nc
    return nc


def _in_maps(lines, videos, video_mask, w_q, w_k):
    w_q = np.asarray(w_q, dtype=np.float32)
    w_k = np.asarray(w_k, dtype=np.float32)
    video_mask = np.asarray(video_mask)
    scale = np.float64(D) ** -0.5
    # scores = lines @ (w_q.T @ w_k * scale) @ videos^T; device wants WL[d', d] = W[d, d']
    WL = (scale * (w_k.astype(np.float64).T @ w_q.astype(np.float64))
          ).astype(np.float32)
    mask_bias = np.where(video_mask == 0,
                         np.float32(-50.0), np.float32(0.0)).astype(np.float32)
    videos = np.asarray(videos, dtype=np.float32)
    lines = np.asarray(lines, dtype=np.float32)
    # vbr layout [v, (b, d | 1 1 | maskbias)] per core
    vones = np.concatenate(
        [videos, np.ones((B, V, 2), dtype=np.float32),
         mask_bias[:, :, None]], axis=2).astype(np.float16)
    vones = vones.reshape(N_CORES, BPC, V, VW).transpose(0, 2, 1, 3)
    vones = np.ascontiguousarray(vones.reshape(N_CORES, V, BPC * VW))
    # lT layout [b][p=d%128, (c=d//128, l)] per core
    linesT = lines.transpose(0, 2, 1).astype(np.float16)    # [B, D, L]
    linesT = linesT.reshape(B, KC, 128, L).transpose(0, 2, 1, 3)
    linesT = np.ascontiguousarray(linesT.reshape(N_CORES, BPC, 128, KC * L))
    # vT layout [p=d'%128, (c, b, v)] per core
    videosT = videos.transpose(0, 2, 1).astype(np.float16)  # [B, D, V]
    videosT = videosT.reshape(N_CORES, BPC, KC, 128, V).transpose(0, 3, 2, 1, 4)
    vT = np.ascontiguousarray(videosT.reshape(N_CORES, 128, KC * BPC * V))
    # wl layout [p, (m, c, s)] with wl[p, m, c, s] = WL[c*128+p, m*128+s]
    WLh = np.ascontiguousarray(
        WL.astype(np.float16).reshape(KC, 128, KC, 128)
        .transpose(1, 2, 0, 3).reshape(128, KC * D))
    maps = []
    for c in range(N_CORES):
        maps.append({
            "linesT": linesT[c],
            "vT": vT[c],
            "vones": vones[c],
            "wl": WLh,
        })
    return maps


def kernel(lines, videos, video_mask, w_q, w_k):
    nc = _build()
    maps = _in_maps(lines, videos, video_mask, w_q, w_k)
    res = run_bass_kernel_spmd(nc, maps, list(range(N_CORES)))
    out = np.concatenate([res.results[c]["out"] for c in range(N_CORES)], axis=0)
    return np.ascontiguousarray(out.astype(np.float32))
